# revision 4
# baseline (speedup 1.0000x reference)
"""Trainium2 Bass kernel for LocallyConnected2d (3x3, pad 1, unshared weights).

  out[b,o,h,w] = sum_{c,k} patches[b,c,k,h,w] * weight[0,o,c,h,w,k]
  x: [8,32,64,64] f32, weight: [1,64,32,64,64,9] f32 -> out: [8,64,64,64] f32

Sharding: H dim split across 8 cores (8 rows each). Each core's weight shard
(36 MiB) is streamed through the TensorE as the moving matmul operand; the
3x3 patches (built on-device from a host-padded x halo) are the stationary
operand. Per spatial location: out[b,:] (8x64) = P_l[288,8]^T @ W_l[288,64],
split into 3 chunks of K=96 accumulating in PSUM, with 4 locations packed
into one PSUM bank via TensorE column tiling (tile_position=(0,32t)).

Host-side relayout of the weight gives the device perfectly contiguous
DMA streams; the kernel is HBM-bandwidth bound (~36 MiB/core).
"""

import os
import sys

sys.path.insert(0, "/opt/trn_rl_repo")

from contextlib import ExitStack

import numpy as np

import concourse.bass as bass  # noqa: F401
import concourse.tile as tile
from concourse import bacc, mybir
from concourse.bass_utils import run_bass_kernel_spmd

F32 = mybir.dt.float32
BF16 = mybir.dt.bfloat16

B, C, O, H, W, K = 8, 32, 64, 64, 64, 9
NCORES = 8
HL = H // NCORES          # 8 spatial rows per core
LOCS = HL * W             # 512 locations per core
NJ = 3                    # contraction chunks (96 = 32c x 3k each)
GL = 32                   # locations per W-DMA group
NG = LOCS // GL           # 16 groups
NSUB = GL // 4            # 8 sub-groups of 4 locations (one PSUM bank each)

_CACHED = {}


def _build_nc(sim: bool = False, repeat: int = 1, variant: str = "full"):
    nc = bacc.Bacc("TRN2", target_bir_lowering=False, debug=False,
                   num_devices=NCORES)
    w_d = nc.dram_tensor("w", [NJ, 96, LOCS, O], BF16,
                         kind="ExternalInput").ap()
    x_d = nc.dram_tensor("x", [C, B, HL + 2, W + 2], F32,
                         kind="ExternalInput").ap()
    # out[t, b, g, sub, o] with location l = g*32 + sub*4 + t
    o_d = nc.dram_tensor("out", [4, B, NG, NSUB, O], F32,
                         kind="ExternalOutput").ap()

    with tile.TileContext(nc) as tc, ExitStack() as ctx:
        xpool = ctx.enter_context(tc.tile_pool(name="xpool", bufs=1))
        ppool = ctx.enter_context(tc.tile_pool(name="ppool", bufs=1))
        wpool = ctx.enter_context(tc.tile_pool(name="wpool", bufs=2))
        pspool = ctx.enter_context(tc.tile_pool(name="pspool", bufs=8,
                                                space="PSUM"))
        stpool = ctx.enter_context(tc.tile_pool(name="stpool", bufs=3))

        x_sb = xpool.tile([C, B, HL + 2, W + 2], F32, name="x_sb")
        nc.sync.dma_start(x_sb[:], x_d[:])

        # Patches: P[j][32*k_in + c, b, h, w] = x[c, b, h + k//3, w + k%3]
        # (k = 3j + k_in; offsets already include the +1 pad shift)
        P = []
        for j in range(NJ):
            Pj = ppool.tile([96, B, HL, W], BF16, name=f"P{j}")
            P.append(Pj)
            for k_in in range(3):
                k = 3 * j + k_in
                dh, dw = k // 3, k % 3
                nc.vector.tensor_copy(
                    Pj[32 * k_in: 32 * (k_in + 1)],
                    x_sb[:, :, dh: dh + HL, dw: dw + W],
                )

        def body():
            for g in range(NG):
                Wg = []
                for j in range(NJ):
                    Wj = wpool.tile([96, GL, O], BF16, name=f"Wt{j}",
                                    tag=f"Wt{j}")
                    Wg.append(Wj)
                    nc.sync.dma_start(Wj[:], w_d[j, :, g * GL:(g + 1) * GL, :])

                stage = stpool.tile([128, NSUB, O], F32, name="stage")
                for sub in range(NSUB):
                    ps = pspool.tile([128, O], F32, name="ps")
                    if sim:
                        nc.vector.memset(ps[:], 0)
                    njs = {"full": NJ, "mm1": 1, "dma": 0}[variant]
                    for j in range(njs):
                        for t in range(4):
                            li = sub * 4 + t
                            l = g * GL + li
                            nc.tensor.matmul(
                                ps[32 * t: 32 * t + B, :],
                                P[j][:, :, l // W, l % W],
                                Wg[j][:, li, :],
                                start=(j == 0),
                                stop=(j == njs - 1),
                                skip_group_check=True,
                                tile_position=(0, 32 * t),
                            )
                    nc.vector.tensor_copy(stage[:, sub, :], ps[:])

                for t in range(4):
                    nc.sync.dma_start(o_d[t, :, g], stage[32 * t: 32 * t + B])

        if repeat > 1:
            with tc.For_i(0, repeat, 1):
                body()
        else:
            body()

    nc.compile()
    return nc


def _shard(x: np.ndarray, weight: np.ndarray):
    # Device weight layout: w[j, p, l, o] with p = 32*k_in + c, k = 3j + k_in,
    # l = h_local*W + w.
    import ml_dtypes

    wt = weight[0].transpose(4, 1, 2, 3, 0)          # [K, C, H, W, O]
    wt = np.ascontiguousarray(wt).astype(ml_dtypes.bfloat16)
    wt = wt.reshape(NJ, 96, H, W, O)
    xp = np.pad(x, ((0, 0), (0, 0), (1, 1), (1, 1))).transpose(1, 0, 2, 3)
    xp = np.ascontiguousarray(xp)                    # [C, B, H+2, W+2]
    in_maps = []
    for i in range(NCORES):
        h0 = i * HL
        in_maps.append({
            "w": np.ascontiguousarray(wt[:, :, h0:h0 + HL]).reshape(
                NJ, 96, LOCS, O),
            "x": np.ascontiguousarray(xp[:, :, h0:h0 + HL + 2, :]),
        })
    return in_maps


def _gather(outs):
    full = np.empty((B, O, H, W), np.float32)
    for i, oc in enumerate(outs):                    # oc [4, B, NG, NSUB, O]
        tmp = oc.transpose(1, 2, 3, 0, 4).reshape(B, LOCS, O)   # [b, l, o]
        tmp = tmp.reshape(B, HL, W, O).transpose(0, 3, 1, 2)    # [b, o, h, w]
        full[:, :, i * HL:(i + 1) * HL, :] = tmp
    return full


def _get_nc():
    if "nc" not in _CACHED:
        _CACHED["nc"] = _build_nc()
    return _CACHED["nc"]


def kernel(**inputs) -> np.ndarray:
    x = np.ascontiguousarray(np.asarray(inputs["x"], dtype=np.float32))
    weight = np.asarray(inputs["weight"], dtype=np.float32)
    in_maps = _shard(x, weight)
    nc = _get_nc()
    res = run_bass_kernel_spmd(nc, in_maps, core_ids=list(range(NCORES)),
                               trace=bool(os.environ.get("BASS_TRACE_RUN")))
    if os.environ.get("BASS_TRACE_RUN"):
        _CACHED["last_results"] = res
    return _gather([r["out"] for r in res.results])

